# revision 37
# baseline (speedup 1.0000x reference)
"""Low-rank (CPD) 3D conv kernel for Trainium2, SPMD across 8 NeuronCores.

Math (per reference):
  y[r,h,w,d]  = sum_c U_c_in[c,r] * x[c,h,w,d]
  z           = conv_h/w/d separable 3-tap convs with per-rank taps U_k*
  out[c,...]  = sum_r U_c_out[r,c] * z[r,...] + bias[c]

Distribution: data-parallel split of H (64) into 8 slabs of 8 planes; each
core reads its slab plus one halo plane on each side (zero at global edges)
and computes its output slab independently. No collectives.

Per-core pipeline, software-pipelined over the 8 output planes (mm2 for
plane h is emitted during iteration h+1 so the tensor engine never stalls
on the conv chain):
  - mm1 with conv_h folded: 3 weight matrices W_k = U_c_in * U_kh[k] (host
    precomputed, bf16); PSUM accumulation over 2 c-tiles x 3 h-taps.
  - ACT drains PSUM twice, casting to bf16: zc = U_kw[1]*y and t0 =
    U_kw[0]*y (the conv_w center + left-tap scalings ride the drain for
    free).
  - conv_w on DVE: t2 = (U_kw[2]/U_kw[1])*zc, then two in-place shifted
    adds (+-1 w-line = +-64 elements, 4B-aligned so DVE runs in fast mode).
  - conv_d: 3 DVE scale muls; the +-1-element d-shifts are 2-byte-misaligned
    (cripples DVE) so they are done as flat DMA shifted copies (1
    descriptor/partition, issued from the GpSimd sequencer - no engine
    contention) + DVE memset of the contaminated d-edge columns + two
    aligned full-plane DVE adds.
  - mm2: lhsT = U_c_out (bf16), accumulate 2 r-tiles; ACT drain adds bias
    and writes bf16 (host upcasts to f32).

GpSimd ALU ops are avoided entirely: concurrent GpSimd execution disables
DVE's 2-port fast modes (5-7x slowdown measured).
"""

import numpy as np
import ml_dtypes

BF16 = ml_dtypes.bfloat16

# Problem constants (hardcoded per contest contract)
C = 256   # input channels
R = 256   # rank
CO = 256  # output channels
S = 64    # spatial extent (cube)
NCORES = 8
HP = S // NCORES          # output planes per core (8)
HS = HP + 2               # slab planes incl. halo (10)
PLANE = S * S             # 4096 elements per (w,d) plane

_cache = {}


def _build_program(hp=HP):
    import concourse.bass as bass
    import concourse.mybir as mybir
    import concourse.tile as tile
    from concourse import bacc

    HS_, HP_ = hp + 2, hp

    fp32 = mybir.dt.float32
    bf16 = mybir.dt.bfloat16

    nc = bacc.Bacc("TRN2", target_bir_lowering=False, debug=False,
                   num_devices=NCORES)

    x_d = nc.dram_tensor("xs", [2, 128, HS_, PLANE], bf16, kind="ExternalInput").ap()
    # all matmul weights packed as one [128, 16*128] bf16 tensor:
    # cols (k*4+ct*2+rt)*128 for wkh, then (12+rt*2+co)*128 for uco
    wts_d = nc.dram_tensor("wts", [128, 16 * 128], bf16, kind="ExternalInput").ap()
    # per-partition scalars packed: [rt0|rt1] x [ukw(3), ukd(3), bias(1)]
    scal_d = nc.dram_tensor("scal", [128, 2, 7], fp32, kind="ExternalInput").ap()
    out_d = nc.dram_tensor("out", [2, 128, HP_, PLANE], bf16, kind="ExternalOutput").ap()

    mult = mybir.AluOpType.mult
    add = mybir.AluOpType.add
    ident = mybir.ActivationFunctionType.Identity

    with tile.TileContext(nc) as tc:
        consts = tc.alloc_tile_pool(name="consts", bufs=1)
        xpool = tc.alloc_tile_pool(name="x", bufs=16)
        zcpool = tc.alloc_tile_pool(name="zc", bufs=4)
        tpool = tc.alloc_tile_pool(name="tmp", bufs=6)
        zdpool = tc.alloc_tile_pool(name="zd", bufs=10)
        opool = tc.alloc_tile_pool(name="osb", bufs=2)
        ps1 = tc.alloc_tile_pool(name="ps1", bufs=3, space="PSUM")
        ps2 = tc.alloc_tile_pool(name="ps2", bufs=2, space="PSUM")

        # ---- constants (2 DMAs total to keep startup issue latency low) ----
        wts = consts.tile([128, 16 * 128], bf16, name="wts", tag="wts")
        nc.sync.dma_start(out=wts, in_=wts_d)
        wkh = [[[wts[:, (k * 4 + ct * 2 + rt) * 128:(k * 4 + ct * 2 + rt + 1) * 128]
                 for rt in range(2)] for ct in range(2)] for k in range(3)]
        uco = [[wts[:, (12 + rt * 2 + co) * 128:(12 + rt * 2 + co + 1) * 128]
                for co in range(2)] for rt in range(2)]
        scal = consts.tile([128, 2, 7], fp32, name="scal", tag="scal")
        nc.sync.dma_start(out=scal, in_=scal_d)
        ukw = [scal[:, rt, 0:3] for rt in range(2)]
        ukd = [scal[:, rt, 3:6] for rt in range(2)]
        bia = [scal[:, co, 6:7] for co in range(2)]

        # ---- x plane streaming (half-plane tiles: finer DMA/mm1 overlap) ----
        xt = {}

        def get_x(p, ct, hf):
            if (p, ct, hf) not in xt:
                t = xpool.tile([128, PLANE // 2], bf16, name="xplane", tag="xplane")
                nc.sync.dma_start(out=t, in_=x_d[ct, :, p,
                                               hf * 2048:(hf + 1) * 2048])
                xt[(p, ct, hf)] = t
            return xt[(p, ct, hf)]

        NQ = PLANE // 1024  # 1024-wide psum tiles per plane

        def mm2_stage(h, zd):
            # mm2 + bias drain (bf16 out; host upcasts). zd is a list of
            # half-plane tiles [rt][hf] of width 2048 so mm2's first chunks
            # start as soon as the first halves are written.
            for co in range(2):
                osb = opool.tile([128, PLANE], bf16, name="osb", tag="osb")
                for q in range(2 * NQ):
                    pt = ps2.tile([128, 512], fp32, name="pt2", tag="ps2")
                    hf = q // 4
                    qo = (q % 4) * 512
                    for rt in range(2):
                        nc.tensor.matmul(
                            pt, uco[rt][co],
                            zd[rt][hf][:, qo:qo + 512],
                            start=(rt == 0),
                            stop=(rt == 1),
                        )
                    nc.scalar.activation(osb[:, q * 512:(q + 1) * 512], pt,
                                         ident, bias=bia[co])
                nc.sync.dma_start(out=out_d[co, :, h, :], in_=osb)

        zd_hist = []
        for h in range(HP_):
            zd = []
            for rt in range(2):
                # --- mm1 + conv_h fold ---
                # Scale-early: the drain scale is Uw1*Ud1, so after conv_w
                # the tile already holds z' = Ud1*conv_w(y) and conv_d's
                # center mul disappears.
                zc = zcpool.tile([128, PLANE], bf16, name="zc", tag="zc")
                t0 = tpool.tile([128, PLANE], bf16, name="t0t", tag="tmp")
                for q in range(NQ):
                    pt = ps1.tile([128, 1024], fp32, name="pt1", tag="ps1")
                    hf, qo = q // 2, (q % 2) * 1024
                    for half in range(2):
                        first = True
                        for k in range(3):
                            for ct in range(2):
                                nc.tensor.matmul(
                                    pt[:, half * 512:(half + 1) * 512],
                                    wkh[k][ct][rt],
                                    get_x(h + k, ct, hf)[:, qo + half * 512:
                                                         qo + (half + 1) * 512],
                                    start=first,
                                    stop=(k == 2 and ct == 1),
                                )
                                first = False
                    # ACT drain: zc = (Uw1*Ud1)*y (f32 PSUM -> bf16)
                    dst = slice(q * 1024, (q + 1) * 1024)
                    nc.scalar.mul(zc[:, dst], pt, ukw[rt][:, 1:2])

                # --- conv_w (DVE; +-64-element shifts, all aligned) ---
                t2 = tpool.tile([128, PLANE], bf16, name="t2t", tag="tmp")
                nc.vector.tensor_scalar_mul(t0, zc, ukw[rt][:, 0:1])
                nc.vector.tensor_scalar_mul(t2, zc, ukw[rt][:, 2:3])
                # in-place: zc becomes z' = Ud1*conv_w(y)
                nc.vector.tensor_tensor(zc[:, 64:], t0[:, :PLANE - 64], zc[:, 64:], add)
                nc.vector.tensor_tensor(zc[:, :PLANE - 64], t2[:, 64:], zc[:, :PLANE - 64], add)

                # --- conv_d: zd = z' + shift(Ud0/Ud1*z') + shift(Ud2/Ud1*z') ---
                a0 = tpool.tile([128, PLANE], bf16, name="a0t", tag="tmp")
                a2 = tpool.tile([128, PLANE], bf16, name="a2t", tag="tmp")
                nc.vector.tensor_scalar_mul(a0, zc, ukd[rt][:, 0:1])
                nc.vector.tensor_scalar_mul(a2, zc, ukd[rt][:, 2:3])
                # flat +-1-element shifts via HWDGE DMA (no DVE misalignment
                # penalty, no GpSimd involvement)
                a0s = tpool.tile([128, PLANE], bf16, name="a0s", tag="tmp")
                a2s = tpool.tile([128, PLANE], bf16, name="a2s", tag="tmp")
                nc.sync.dma_start(out=a0s[:, 1:], in_=a0[:, :PLANE - 1])
                nc.sync.dma_start(out=a2s[:, :PLANE - 1], in_=a2[:, 1:])
                # zero the cross-w-line contaminated columns (d=0 / d=63)
                a0v = a0s.rearrange("p (w d) -> p w d", d=64)
                a2v = a2s.rearrange("p (w d) -> p w d", d=64)
                nc.vector.memset(a0v[:, :, 0:1], 0.0)
                nc.vector.memset(a2v[:, :, 63:64], 0.0)
                # zd as two half-plane tiles for finer mm2 pipelining
                zh = []
                for hf in range(2):
                    sl = slice(hf * 2048, (hf + 1) * 2048)
                    zt = zdpool.tile([128, 2048], bf16, name="zdt", tag="zd")
                    nc.vector.tensor_tensor(zt, a0s[:, sl], zc[:, sl], add)
                    nc.vector.tensor_tensor(zt, a2s[:, sl], zt, add)
                    zh.append(zt)
                zd.append(zh)

            # software pipelining: mm2 lags TWO iterations so the tensor
            # engine never catches up with the DVE conv chain.
            zd_hist.append(zd)
            if h >= 1:
                mm2_stage(h - 1, zd_hist[h - 1])

        mm2_stage(HP_ - 1, zd_hist[HP_ - 1])

        for pool in (ps2, ps1, opool, zdpool, tpool, zcpool, xpool, consts):
            pool.release()

    nc.compile()
    return nc


def _host_prep(x, U_kh, U_kw, U_kd, U_c_in, U_c_out, bias):
    """Build per-core input maps (numpy only)."""
    x = np.asarray(x)
    U_kh = np.asarray(U_kh, np.float32)
    U_kw = np.asarray(U_kw, np.float32)
    U_kd = np.asarray(U_kd, np.float32)
    U_c_in = np.asarray(U_c_in, np.float32)
    U_c_out = np.asarray(U_c_out, np.float32)
    bias = np.asarray(bias, np.float32)

    xb = np.ascontiguousarray(x[0]).astype(BF16)          # [C, S, S, S]
    xb = xb.reshape(C, S, PLANE)

    # Packed weights [128, 16*128]: wkh at col-block k*4+ct*2+rt, uco at
    # 12+rt*2+co.
    wts = np.empty((128, 16 * 128), BF16)
    for k in range(3):
        wk = (U_c_in * U_kh[k][None, :]).astype(BF16)     # [C, R]
        wk4 = wk.reshape(2, 128, 2, 128)                  # [ct,128c,rt,128r]
        for ct in range(2):
            for rt in range(2):
                b = k * 4 + ct * 2 + rt
                wts[:, b * 128:(b + 1) * 128] = wk4[ct, :, rt, :]
    uco4 = U_c_out.astype(BF16).reshape(2, 128, 2, 128)   # [rt,128r,co,128c]
    for rt in range(2):
        for co in range(2):
            b = 12 + rt * 2 + co
            wts[:, b * 128:(b + 1) * 128] = uco4[rt, :, co, :]

    # Scale-early scalars (Uw1/Ud1 clamped away from 0):
    #   drain scale  s1  = Uw1*Ud1        -> z' = Ud1*conv_w(y) after adds
    #   t0 scale: rt0 (ACT, from psum) Uw0*Ud1 ; rt1 (DVE, from zc') Uw0/Uw1
    #   t2 ratio  Uw2/Uw1 ; conv_d ratios Ud0/Ud1, Ud2/Ud1
    tiny = np.float32(1e-6)
    uw0, uw1, uw2 = (U_kw[k].astype(np.float32).copy() for k in range(3))
    ud0, ud1, ud2 = (U_kd[k].astype(np.float32).copy() for k in range(3))
    uw1[np.abs(uw1) < tiny] = tiny
    ud1[np.abs(ud1) < tiny] = tiny
    uw = np.empty((R, 3), np.float32)
    uw[:, 1] = uw1 * ud1
    uw[:, 0] = uw0 / uw1                  # DVE t0 ratio from zc'
    uw[:, 2] = uw2 / uw1
    ud = np.empty((R, 3), np.float32)
    ud[:, 0] = ud0 / ud1
    ud[:, 1] = ud1
    ud[:, 2] = ud2 / ud1
    # scal [128, 2, 7]: per rank-tile [ukw(3), ukd(3)], col 6 = bias per
    # CO-tile.
    scal = np.empty((128, 2, 7), np.float32)
    uwr = uw.reshape(2, 128, 3)
    udr = ud.reshape(2, 128, 3)
    for t in range(2):
        scal[:, t, 0:3] = uwr[t]
        scal[:, t, 3:6] = udr[t]
        scal[:, t, 6] = bias.reshape(2, 128)[t]

    in_maps = []
    for c in range(NCORES):
        slab = np.zeros((C, HS, PLANE), BF16)
        lo, hi = c * HP - 1, c * HP + HP + 1
        s0, s1 = max(lo, 0), min(hi, S)
        slab[:, s0 - lo:HS - (hi - s1)] = xb[:, s0:s1]
        slab = np.ascontiguousarray(slab.reshape(2, 128, HS, PLANE))
        in_maps.append({"xs": slab, "wts": wts, "scal": scal})
    return in_maps


def kernel(x, U_kh, U_kw, U_kd, U_c_in, U_c_out, bias, _trace=False):
    from concourse.bass_utils import run_bass_kernel_spmd

    if "nc" not in _cache:
        _cache["nc"] = _build_program()
    nc = _cache["nc"]

    in_maps = _host_prep(x, U_kh, U_kw, U_kd, U_c_in, U_c_out, bias)
    res = run_bass_kernel_spmd(nc, in_maps, core_ids=list(range(NCORES)),
                               trace=_trace)
    _cache["last_result"] = res

    out = np.empty((1, CO, S, S, S), np.float32)
    for c in range(NCORES):
        o = res.results[c]["out"]                        # [2, 128, HP, PLANE] bf16
        out[0, :, c * HP:(c + 1) * HP] = o.reshape(CO, HP, S, S).astype(np.float32)
    return out


# revision 38
# speedup vs baseline: 1.0130x; 1.0130x over previous
"""Low-rank (CPD) 3D conv kernel for Trainium2, SPMD across 8 NeuronCores.

Math (per reference):
  y[r,h,w,d]  = sum_c U_c_in[c,r] * x[c,h,w,d]
  z           = conv_h/w/d separable 3-tap convs with per-rank taps U_k*
  out[c,...]  = sum_r U_c_out[r,c] * z[r,...] + bias[c]

Distribution: data-parallel split of H (64) into 8 slabs of 8 planes; each
core reads its slab plus one halo plane on each side (zero at global edges)
and computes its output slab independently. No collectives.

Per-core pipeline, software-pipelined over the 8 output planes (mm2 for
plane h is emitted during iteration h+1 so the tensor engine never stalls
on the conv chain):
  - mm1 with conv_h folded: 3 weight matrices W_k = U_c_in * U_kh[k] (host
    precomputed, bf16); PSUM accumulation over 2 c-tiles x 3 h-taps.
  - ACT drains PSUM twice, casting to bf16: zc = U_kw[1]*y and t0 =
    U_kw[0]*y (the conv_w center + left-tap scalings ride the drain for
    free).
  - conv_w on DVE: t2 = (U_kw[2]/U_kw[1])*zc, then two in-place shifted
    adds (+-1 w-line = +-64 elements, 4B-aligned so DVE runs in fast mode).
  - conv_d: 3 DVE scale muls; the +-1-element d-shifts are 2-byte-misaligned
    (cripples DVE) so they are done as flat DMA shifted copies (1
    descriptor/partition, issued from the GpSimd sequencer - no engine
    contention) + DVE memset of the contaminated d-edge columns + two
    aligned full-plane DVE adds.
  - mm2: lhsT = U_c_out (bf16), accumulate 2 r-tiles; ACT drain adds bias
    and writes bf16 (host upcasts to f32).

GpSimd ALU ops are avoided entirely: concurrent GpSimd execution disables
DVE's 2-port fast modes (5-7x slowdown measured).
"""

import numpy as np
import ml_dtypes

BF16 = ml_dtypes.bfloat16

# Problem constants (hardcoded per contest contract)
C = 256   # input channels
R = 256   # rank
CO = 256  # output channels
S = 64    # spatial extent (cube)
NCORES = 8
HP = S // NCORES          # output planes per core (8)
HS = HP + 2               # slab planes incl. halo (10)
PLANE = S * S             # 4096 elements per (w,d) plane

_cache = {}


def _build_program(hp=HP):
    import concourse.bass as bass
    import concourse.mybir as mybir
    import concourse.tile as tile
    from concourse import bacc

    HS_, HP_ = hp + 2, hp

    fp32 = mybir.dt.float32
    bf16 = mybir.dt.bfloat16

    nc = bacc.Bacc("TRN2", target_bir_lowering=False, debug=False,
                   num_devices=NCORES)

    x_d = nc.dram_tensor("xs", [2, 128, HS_, PLANE], bf16, kind="ExternalInput").ap()
    # all matmul weights packed as one [128, 16*128] bf16 tensor:
    # cols (k*4+ct*2+rt)*128 for wkh, then (12+rt*2+co)*128 for uco
    wts_d = nc.dram_tensor("wts", [128, 16 * 128], bf16, kind="ExternalInput").ap()
    # per-partition scalars packed: [rt0|rt1] x [ukw(3), ukd(3), bias(1)]
    scal_d = nc.dram_tensor("scal", [128, 2, 7], fp32, kind="ExternalInput").ap()
    out_d = nc.dram_tensor("out", [2, 128, HP_, PLANE], bf16, kind="ExternalOutput").ap()

    mult = mybir.AluOpType.mult
    add = mybir.AluOpType.add
    ident = mybir.ActivationFunctionType.Identity

    with tile.TileContext(nc) as tc:
        consts = tc.alloc_tile_pool(name="consts", bufs=1)
        xpool = tc.alloc_tile_pool(name="x", bufs=16)
        zcpool = tc.alloc_tile_pool(name="zc", bufs=4)
        tpool = tc.alloc_tile_pool(name="tmp", bufs=6)
        zdpool = tc.alloc_tile_pool(name="zd", bufs=10)
        opool = tc.alloc_tile_pool(name="osb", bufs=2)
        ps1 = tc.alloc_tile_pool(name="ps1", bufs=2, space="PSUM")
        ps2 = tc.alloc_tile_pool(name="ps2", bufs=2, space="PSUM")

        # ---- constants (2 DMAs total to keep startup issue latency low) ----
        wts = consts.tile([128, 16 * 128], bf16, name="wts", tag="wts")
        nc.sync.dma_start(out=wts, in_=wts_d)
        wkh = [[[wts[:, (k * 4 + ct * 2 + rt) * 128:(k * 4 + ct * 2 + rt + 1) * 128]
                 for rt in range(2)] for ct in range(2)] for k in range(3)]
        uco = [[wts[:, (12 + rt * 2 + co) * 128:(12 + rt * 2 + co + 1) * 128]
                for co in range(2)] for rt in range(2)]
        scal = consts.tile([128, 2, 7], fp32, name="scal", tag="scal")
        nc.sync.dma_start(out=scal, in_=scal_d)
        ukw = [scal[:, rt, 0:3] for rt in range(2)]
        ukd = [scal[:, rt, 3:6] for rt in range(2)]
        bia = [scal[:, co, 6:7] for co in range(2)]

        # ---- x plane streaming (half-plane tiles: finer DMA/mm1 overlap) ----
        xt = {}

        def get_x(p, ct, hf):
            if (p, ct, hf) not in xt:
                t = xpool.tile([128, PLANE // 2], bf16, name="xplane", tag="xplane")
                nc.sync.dma_start(out=t, in_=x_d[ct, :, p,
                                               hf * 2048:(hf + 1) * 2048])
                xt[(p, ct, hf)] = t
            return xt[(p, ct, hf)]

        NQ = PLANE // 1024  # 1024-wide psum tiles per plane

        def mm2_stage(h, zd):
            # mm2 + bias drain (bf16 out; host upcasts). zd is a list of
            # half-plane tiles [rt][hf] of width 2048 so mm2's first chunks
            # start as soon as the first halves are written.
            for co in range(2):
                osb = opool.tile([128, PLANE], bf16, name="osb", tag="osb")
                for q in range(NQ):
                    pt = ps2.tile([128, 1024], fp32, name="pt2", tag="ps2")
                    hf = q // 2
                    qo = (q % 2) * 1024
                    for half in range(2):
                        for rt in range(2):
                            nc.tensor.matmul(
                                pt[:, half * 512:(half + 1) * 512],
                                uco[rt][co],
                                zd[rt][hf][:, qo + half * 512:
                                           qo + (half + 1) * 512],
                                start=(rt == 0),
                                stop=(rt == 1),
                            )
                    nc.scalar.activation(osb[:, q * 1024:(q + 1) * 1024], pt,
                                         ident, bias=bia[co])
                nc.sync.dma_start(out=out_d[co, :, h, :], in_=osb)

        zd_hist = []
        for h in range(HP_):
            zd = []
            for rt in range(2):
                # --- mm1 + conv_h fold ---
                # Scale-early: the drain scale is Uw1*Ud1, so after conv_w
                # the tile already holds z' = Ud1*conv_w(y) and conv_d's
                # center mul disappears.
                zc = zcpool.tile([128, PLANE], bf16, name="zc", tag="zc")
                t0 = tpool.tile([128, PLANE], bf16, name="t0t", tag="tmp")
                for q in range(NQ):
                    pt = ps1.tile([128, 1024], fp32, name="pt1", tag="ps1")
                    hf, qo = q // 2, (q % 2) * 1024
                    for half in range(2):
                        first = True
                        for k in range(3):
                            for ct in range(2):
                                nc.tensor.matmul(
                                    pt[:, half * 512:(half + 1) * 512],
                                    wkh[k][ct][rt],
                                    get_x(h + k, ct, hf)[:, qo + half * 512:
                                                         qo + (half + 1) * 512],
                                    start=first,
                                    stop=(k == 2 and ct == 1),
                                )
                                first = False
                    # ACT drain: zc = (Uw1*Ud1)*y (f32 PSUM -> bf16)
                    dst = slice(q * 1024, (q + 1) * 1024)
                    nc.scalar.mul(zc[:, dst], pt, ukw[rt][:, 1:2])

                # --- conv_w (DVE; +-64-element shifts, all aligned) ---
                t2 = tpool.tile([128, PLANE], bf16, name="t2t", tag="tmp")
                nc.vector.tensor_scalar_mul(t0, zc, ukw[rt][:, 0:1])
                nc.vector.tensor_scalar_mul(t2, zc, ukw[rt][:, 2:3])
                # in-place: zc becomes z' = Ud1*conv_w(y)
                nc.vector.tensor_tensor(zc[:, 64:], t0[:, :PLANE - 64], zc[:, 64:], add)
                nc.vector.tensor_tensor(zc[:, :PLANE - 64], t2[:, 64:], zc[:, :PLANE - 64], add)

                # --- conv_d: zd = z' + shift(Ud0/Ud1*z') + shift(Ud2/Ud1*z') ---
                a0 = tpool.tile([128, PLANE], bf16, name="a0t", tag="tmp")
                a2 = tpool.tile([128, PLANE], bf16, name="a2t", tag="tmp")
                nc.vector.tensor_scalar_mul(a0, zc, ukd[rt][:, 0:1])
                nc.vector.tensor_scalar_mul(a2, zc, ukd[rt][:, 2:3])
                # flat +-1-element shifts via HWDGE DMA (no DVE misalignment
                # penalty, no GpSimd involvement)
                a0s = tpool.tile([128, PLANE], bf16, name="a0s", tag="tmp")
                a2s = tpool.tile([128, PLANE], bf16, name="a2s", tag="tmp")
                nc.sync.dma_start(out=a0s[:, 1:], in_=a0[:, :PLANE - 1])
                nc.sync.dma_start(out=a2s[:, :PLANE - 1], in_=a2[:, 1:])
                # zero the cross-w-line contaminated columns (d=0 / d=63)
                a0v = a0s.rearrange("p (w d) -> p w d", d=64)
                a2v = a2s.rearrange("p (w d) -> p w d", d=64)
                nc.vector.memset(a0v[:, :, 0:1], 0.0)
                nc.vector.memset(a2v[:, :, 63:64], 0.0)
                # zd as two half-plane tiles for finer mm2 pipelining
                zh = []
                for hf in range(2):
                    sl = slice(hf * 2048, (hf + 1) * 2048)
                    zt = zdpool.tile([128, 2048], bf16, name="zdt", tag="zd")
                    nc.vector.tensor_tensor(zt, a0s[:, sl], zc[:, sl], add)
                    nc.vector.tensor_tensor(zt, a2s[:, sl], zt, add)
                    zh.append(zt)
                zd.append(zh)

            # software pipelining: mm2 lags TWO iterations so the tensor
            # engine never catches up with the DVE conv chain.
            zd_hist.append(zd)
            if h >= 1:
                mm2_stage(h - 1, zd_hist[h - 1])

        mm2_stage(HP_ - 1, zd_hist[HP_ - 1])

        for pool in (ps2, ps1, opool, zdpool, tpool, zcpool, xpool, consts):
            pool.release()

    nc.compile()
    return nc


def _host_prep(x, U_kh, U_kw, U_kd, U_c_in, U_c_out, bias):
    """Build per-core input maps (numpy only)."""
    x = np.asarray(x)
    U_kh = np.asarray(U_kh, np.float32)
    U_kw = np.asarray(U_kw, np.float32)
    U_kd = np.asarray(U_kd, np.float32)
    U_c_in = np.asarray(U_c_in, np.float32)
    U_c_out = np.asarray(U_c_out, np.float32)
    bias = np.asarray(bias, np.float32)

    xb = np.ascontiguousarray(x[0]).astype(BF16)          # [C, S, S, S]
    xb = xb.reshape(C, S, PLANE)

    # Packed weights [128, 16*128]: wkh at col-block k*4+ct*2+rt, uco at
    # 12+rt*2+co.
    wts = np.empty((128, 16 * 128), BF16)
    for k in range(3):
        wk = (U_c_in * U_kh[k][None, :]).astype(BF16)     # [C, R]
        wk4 = wk.reshape(2, 128, 2, 128)                  # [ct,128c,rt,128r]
        for ct in range(2):
            for rt in range(2):
                b = k * 4 + ct * 2 + rt
                wts[:, b * 128:(b + 1) * 128] = wk4[ct, :, rt, :]
    uco4 = U_c_out.astype(BF16).reshape(2, 128, 2, 128)   # [rt,128r,co,128c]
    for rt in range(2):
        for co in range(2):
            b = 12 + rt * 2 + co
            wts[:, b * 128:(b + 1) * 128] = uco4[rt, :, co, :]

    # Scale-early scalars (Uw1/Ud1 clamped away from 0):
    #   drain scale  s1  = Uw1*Ud1        -> z' = Ud1*conv_w(y) after adds
    #   t0 scale: rt0 (ACT, from psum) Uw0*Ud1 ; rt1 (DVE, from zc') Uw0/Uw1
    #   t2 ratio  Uw2/Uw1 ; conv_d ratios Ud0/Ud1, Ud2/Ud1
    tiny = np.float32(1e-6)
    uw0, uw1, uw2 = (U_kw[k].astype(np.float32).copy() for k in range(3))
    ud0, ud1, ud2 = (U_kd[k].astype(np.float32).copy() for k in range(3))
    uw1[np.abs(uw1) < tiny] = tiny
    ud1[np.abs(ud1) < tiny] = tiny
    uw = np.empty((R, 3), np.float32)
    uw[:, 1] = uw1 * ud1
    uw[:, 0] = uw0 / uw1                  # DVE t0 ratio from zc'
    uw[:, 2] = uw2 / uw1
    ud = np.empty((R, 3), np.float32)
    ud[:, 0] = ud0 / ud1
    ud[:, 1] = ud1
    ud[:, 2] = ud2 / ud1
    # scal [128, 2, 7]: per rank-tile [ukw(3), ukd(3)], col 6 = bias per
    # CO-tile.
    scal = np.empty((128, 2, 7), np.float32)
    uwr = uw.reshape(2, 128, 3)
    udr = ud.reshape(2, 128, 3)
    for t in range(2):
        scal[:, t, 0:3] = uwr[t]
        scal[:, t, 3:6] = udr[t]
        scal[:, t, 6] = bias.reshape(2, 128)[t]

    in_maps = []
    for c in range(NCORES):
        slab = np.zeros((C, HS, PLANE), BF16)
        lo, hi = c * HP - 1, c * HP + HP + 1
        s0, s1 = max(lo, 0), min(hi, S)
        slab[:, s0 - lo:HS - (hi - s1)] = xb[:, s0:s1]
        slab = np.ascontiguousarray(slab.reshape(2, 128, HS, PLANE))
        in_maps.append({"xs": slab, "wts": wts, "scal": scal})
    return in_maps


def kernel(x, U_kh, U_kw, U_kd, U_c_in, U_c_out, bias, _trace=False):
    from concourse.bass_utils import run_bass_kernel_spmd

    if "nc" not in _cache:
        _cache["nc"] = _build_program()
    nc = _cache["nc"]

    in_maps = _host_prep(x, U_kh, U_kw, U_kd, U_c_in, U_c_out, bias)
    res = run_bass_kernel_spmd(nc, in_maps, core_ids=list(range(NCORES)),
                               trace=_trace)
    _cache["last_result"] = res

    out = np.empty((1, CO, S, S, S), np.float32)
    for c in range(NCORES):
        o = res.results[c]["out"]                        # [2, 128, HP, PLANE] bf16
        out[0, :, c * HP:(c + 1) * HP] = o.reshape(CO, HP, S, S).astype(np.float32)
    return out


# revision 39
# speedup vs baseline: 1.1557x; 1.1409x over previous
"""Low-rank (CPD) 3D conv kernel for Trainium2, SPMD across 8 NeuronCores.

Math (per reference):
  y[r,h,w,d]  = sum_c U_c_in[c,r] * x[c,h,w,d]
  z           = conv_h/w/d separable 3-tap convs with per-rank taps U_k*
  out[c,...]  = sum_r U_c_out[r,c] * z[r,...] + bias[c]

Distribution: data-parallel split of H (64) into 8 slabs of 8 planes; each
core reads its slab plus one halo plane on each side (zero at global edges)
and computes its output slab independently. No collectives.

Per-core pipeline, software-pipelined over the 8 output planes (mm2 for
plane h is emitted during iteration h+1 so the tensor engine never stalls
on the conv chain):
  - mm1 with conv_h folded: 3 weight matrices W_k = U_c_in * U_kh[k] (host
    precomputed, bf16); PSUM accumulation over 2 c-tiles x 3 h-taps.
  - ACT drains PSUM twice, casting to bf16: zc = U_kw[1]*y and t0 =
    U_kw[0]*y (the conv_w center + left-tap scalings ride the drain for
    free).
  - conv_w on DVE: t2 = (U_kw[2]/U_kw[1])*zc, then two in-place shifted
    adds (+-1 w-line = +-64 elements, 4B-aligned so DVE runs in fast mode).
  - conv_d: 3 DVE scale muls; the +-1-element d-shifts are 2-byte-misaligned
    (cripples DVE) so they are done as flat DMA shifted copies (1
    descriptor/partition, issued from the GpSimd sequencer - no engine
    contention) + DVE memset of the contaminated d-edge columns + two
    aligned full-plane DVE adds.
  - mm2: lhsT = U_c_out (bf16), accumulate 2 r-tiles; ACT drain adds bias
    and writes bf16 (host upcasts to f32).

GpSimd ALU ops are avoided entirely: concurrent GpSimd execution disables
DVE's 2-port fast modes (5-7x slowdown measured).
"""

import numpy as np
import ml_dtypes

BF16 = ml_dtypes.bfloat16

# Problem constants (hardcoded per contest contract)
C = 256   # input channels
R = 256   # rank
CO = 256  # output channels
S = 64    # spatial extent (cube)
NCORES = 8
HP = S // NCORES          # output planes per core (8)
HS = HP + 2               # slab planes incl. halo (10)
PLANE = S * S             # 4096 elements per (w,d) plane

_cache = {}


def _build_program(hp=HP):
    import concourse.bass as bass
    import concourse.mybir as mybir
    import concourse.tile as tile
    from concourse import bacc

    HS_, HP_ = hp + 2, hp

    fp32 = mybir.dt.float32
    bf16 = mybir.dt.bfloat16

    nc = bacc.Bacc("TRN2", target_bir_lowering=False, debug=False,
                   num_devices=NCORES)

    x_d = nc.dram_tensor("xs", [2, 128, HS_, PLANE], bf16, kind="ExternalInput").ap()
    # all matmul weights packed as one [128, 16*128] bf16 tensor:
    # cols (k*4+ct*2+rt)*128 for wkh, then (12+rt*2+co)*128 for uco
    wts_d = nc.dram_tensor("wts", [128, 16 * 128], bf16, kind="ExternalInput").ap()
    # per-partition scalars packed: [rt0|rt1] x [ukw(3), ukd(3), bias(1)]
    scal_d = nc.dram_tensor("scal", [128, 2, 7], fp32, kind="ExternalInput").ap()
    d1_d = nc.dram_tensor("d1", [128, 1], fp32, kind="ExternalInput").ap()
    d2_d = nc.dram_tensor("d2", [128, 1], fp32, kind="ExternalInput").ap()
    d3_d = nc.dram_tensor("d3", [128, 1], fp32, kind="ExternalInput").ap()
    out_d = nc.dram_tensor("out", [2, 128, HP_, PLANE], bf16, kind="ExternalOutput").ap()

    mult = mybir.AluOpType.mult
    add = mybir.AluOpType.add
    ident = mybir.ActivationFunctionType.Identity

    with tile.TileContext(nc) as tc:
        consts = tc.alloc_tile_pool(name="consts", bufs=1)
        xpool = tc.alloc_tile_pool(name="x", bufs=16)
        zcpool = tc.alloc_tile_pool(name="zc", bufs=4)
        tpool = tc.alloc_tile_pool(name="tmp", bufs=6)
        zdpool = tc.alloc_tile_pool(name="zd", bufs=10)
        opool = tc.alloc_tile_pool(name="osb", bufs=2)
        ps1 = tc.alloc_tile_pool(name="ps1", bufs=2, space="PSUM")
        ps2 = tc.alloc_tile_pool(name="ps2", bufs=2, space="PSUM")

        # ---- constants (2 DMAs total to keep startup issue latency low) ----
        wts = consts.tile([128, 16 * 128], bf16, name="wts", tag="wts")
        nc.sync.dma_start(out=wts, in_=wts_d)
        wkh = [[[wts[:, (k * 4 + ct * 2 + rt) * 128:(k * 4 + ct * 2 + rt + 1) * 128]
                 for rt in range(2)] for ct in range(2)] for k in range(3)]
        uco = [[wts[:, (12 + rt * 2 + co) * 128:(12 + rt * 2 + co + 1) * 128]
                for co in range(2)] for rt in range(2)]
        scal = consts.tile([128, 2, 7], fp32, name="scal", tag="scal")
        nc.sync.dma_start(out=scal, in_=scal_d)
        dts = [consts.tile([128, 1], fp32, name=f"dt{i}", tag=f"dt{i}") for i in range(3)]
        for t, d in zip(dts, (d1_d, d2_d, d3_d)):
            nc.sync.dma_start(out=t, in_=d)
        ukw = [scal[:, rt, 0:3] for rt in range(2)]
        ukd = [scal[:, rt, 3:6] for rt in range(2)]
        bia = [scal[:, co, 6:7] for co in range(2)]

        # ---- x plane streaming (half-plane tiles: finer DMA/mm1 overlap) ----
        xt = {}

        def get_x(p, ct, hf):
            if (p, ct, hf) not in xt:
                t = xpool.tile([128, PLANE // 2], bf16, name="xplane", tag="xplane")
                nc.sync.dma_start(out=t, in_=x_d[ct, :, p,
                                               hf * 2048:(hf + 1) * 2048])
                xt[(p, ct, hf)] = t
            return xt[(p, ct, hf)]

        NQ = PLANE // 1024  # 1024-wide psum tiles per plane

        def mm2_stage(h, zd):
            # mm2 + bias drain (bf16 out; host upcasts). zd is a list of
            # half-plane tiles [rt][hf] of width 2048 so mm2's first chunks
            # start as soon as the first halves are written.
            for co in range(2):
                osb = opool.tile([128, PLANE], bf16, name="osb", tag="osb")
                for q in range(NQ):
                    pt = ps2.tile([128, 1024], fp32, name="pt2", tag="ps2")
                    hf = q // 2
                    qo = (q % 2) * 1024
                    for half in range(2):
                        for rt in range(2):
                            nc.tensor.matmul(
                                pt[:, half * 512:(half + 1) * 512],
                                uco[rt][co],
                                zd[rt][hf][:, qo + half * 512:
                                           qo + (half + 1) * 512],
                                start=(rt == 0),
                                stop=(rt == 1),
                            )
                    nc.scalar.activation(osb[:, q * 1024:(q + 1) * 1024], pt,
                                         ident, bias=bia[co])
                nc.sync.dma_start(out=out_d[co, :, h, :], in_=osb)

        zd_hist = []
        for h in range(HP_):
            zd = []
            for rt in range(2):
                # --- mm1 + conv_h fold ---
                # Scale-early: the drain scale is Uw1*Ud1, so after conv_w
                # the tile already holds z' = Ud1*conv_w(y) and conv_d's
                # center mul disappears.
                zc = zcpool.tile([128, PLANE], bf16, name="zc", tag="zc")
                t0 = tpool.tile([128, PLANE], bf16, name="t0t", tag="tmp")
                for q in range(NQ):
                    pt = ps1.tile([128, 1024], fp32, name="pt1", tag="ps1")
                    hf, qo = q // 2, (q % 2) * 1024
                    for half in range(2):
                        first = True
                        for k in range(3):
                            for ct in range(2):
                                nc.tensor.matmul(
                                    pt[:, half * 512:(half + 1) * 512],
                                    wkh[k][ct][rt],
                                    get_x(h + k, ct, hf)[:, qo + half * 512:
                                                         qo + (half + 1) * 512],
                                    start=first,
                                    stop=(k == 2 and ct == 1),
                                )
                                first = False
                    # ACT drain: zc = (Uw1*Ud1)*y (f32 PSUM -> bf16)
                    dst = slice(q * 1024, (q + 1) * 1024)
                    nc.scalar.mul(zc[:, dst], pt, ukw[rt][:, 1:2])

                # --- conv_w (DVE; +-64-element shifts, all aligned) ---
                t2 = tpool.tile([128, PLANE], bf16, name="t2t", tag="tmp")
                nc.vector.tensor_scalar_mul(t0, zc, ukw[rt][:, 0:1])
                nc.vector.tensor_scalar_mul(t2, zc, ukw[rt][:, 2:3])
                # in-place: zc becomes z' = Ud1*conv_w(y)
                nc.vector.tensor_tensor(zc[:, 64:], t0[:, :PLANE - 64], zc[:, 64:], add)
                nc.vector.tensor_tensor(zc[:, :PLANE - 64], t2[:, 64:], zc[:, :PLANE - 64], add)

                # --- conv_d: zd = z' + shift(Ud0/Ud1*z') + shift(Ud2/Ud1*z') ---
                a0 = tpool.tile([128, PLANE], bf16, name="a0t", tag="tmp")
                a2 = tpool.tile([128, PLANE], bf16, name="a2t", tag="tmp")
                nc.vector.tensor_scalar_mul(a0, zc, ukd[rt][:, 0:1])
                nc.vector.tensor_scalar_mul(a2, zc, ukd[rt][:, 2:3])
                # flat +-1-element shifts via HWDGE DMA (no DVE misalignment
                # penalty, no GpSimd involvement)
                a0s = tpool.tile([128, PLANE], bf16, name="a0s", tag="tmp")
                a2s = tpool.tile([128, PLANE], bf16, name="a2s", tag="tmp")
                nc.sync.dma_start(out=a0s[:, 1:], in_=a0[:, :PLANE - 1])
                nc.sync.dma_start(out=a2s[:, :PLANE - 1], in_=a2[:, 1:])
                # zero the cross-w-line contaminated columns (d=0 / d=63)
                a0v = a0s.rearrange("p (w d) -> p w d", d=64)
                a2v = a2s.rearrange("p (w d) -> p w d", d=64)
                nc.vector.memset(a0v[:, :, 0:1], 0.0)
                nc.vector.memset(a2v[:, :, 63:64], 0.0)
                # zd as two half-plane tiles for finer mm2 pipelining
                zh = []
                for hf in range(2):
                    sl = slice(hf * 2048, (hf + 1) * 2048)
                    zt = zdpool.tile([128, 2048], bf16, name="zdt", tag="zd")
                    nc.vector.tensor_tensor(zt, a0s[:, sl], zc[:, sl], add)
                    nc.vector.tensor_tensor(zt, a2s[:, sl], zt, add)
                    zh.append(zt)
                zd.append(zh)

            # software pipelining: mm2 lags TWO iterations so the tensor
            # engine never catches up with the DVE conv chain.
            zd_hist.append(zd)
            if h >= 1:
                mm2_stage(h - 1, zd_hist[h - 1])

        mm2_stage(HP_ - 1, zd_hist[HP_ - 1])

        for pool in (ps2, ps1, opool, zdpool, tpool, zcpool, xpool, consts):
            pool.release()

    nc.compile()
    return nc


def _host_prep(x, U_kh, U_kw, U_kd, U_c_in, U_c_out, bias):
    """Build per-core input maps (numpy only)."""
    x = np.asarray(x)
    U_kh = np.asarray(U_kh, np.float32)
    U_kw = np.asarray(U_kw, np.float32)
    U_kd = np.asarray(U_kd, np.float32)
    U_c_in = np.asarray(U_c_in, np.float32)
    U_c_out = np.asarray(U_c_out, np.float32)
    bias = np.asarray(bias, np.float32)

    xb = np.ascontiguousarray(x[0]).astype(BF16)          # [C, S, S, S]
    xb = xb.reshape(C, S, PLANE)

    # Packed weights [128, 16*128]: wkh at col-block k*4+ct*2+rt, uco at
    # 12+rt*2+co.
    wts = np.empty((128, 16 * 128), BF16)
    for k in range(3):
        wk = (U_c_in * U_kh[k][None, :]).astype(BF16)     # [C, R]
        wk4 = wk.reshape(2, 128, 2, 128)                  # [ct,128c,rt,128r]
        for ct in range(2):
            for rt in range(2):
                b = k * 4 + ct * 2 + rt
                wts[:, b * 128:(b + 1) * 128] = wk4[ct, :, rt, :]
    uco4 = U_c_out.astype(BF16).reshape(2, 128, 2, 128)   # [rt,128r,co,128c]
    for rt in range(2):
        for co in range(2):
            b = 12 + rt * 2 + co
            wts[:, b * 128:(b + 1) * 128] = uco4[rt, :, co, :]

    # Scale-early scalars (Uw1/Ud1 clamped away from 0):
    #   drain scale  s1  = Uw1*Ud1        -> z' = Ud1*conv_w(y) after adds
    #   t0 scale: rt0 (ACT, from psum) Uw0*Ud1 ; rt1 (DVE, from zc') Uw0/Uw1
    #   t2 ratio  Uw2/Uw1 ; conv_d ratios Ud0/Ud1, Ud2/Ud1
    tiny = np.float32(1e-6)
    uw0, uw1, uw2 = (U_kw[k].astype(np.float32).copy() for k in range(3))
    ud0, ud1, ud2 = (U_kd[k].astype(np.float32).copy() for k in range(3))
    uw1[np.abs(uw1) < tiny] = tiny
    ud1[np.abs(ud1) < tiny] = tiny
    uw = np.empty((R, 3), np.float32)
    uw[:, 1] = uw1 * ud1
    uw[:, 0] = uw0 / uw1                  # DVE t0 ratio from zc'
    uw[:, 2] = uw2 / uw1
    ud = np.empty((R, 3), np.float32)
    ud[:, 0] = ud0 / ud1
    ud[:, 1] = ud1
    ud[:, 2] = ud2 / ud1
    # scal [128, 2, 7]: per rank-tile [ukw(3), ukd(3)], col 6 = bias per
    # CO-tile.
    scal = np.empty((128, 2, 7), np.float32)
    uwr = uw.reshape(2, 128, 3)
    udr = ud.reshape(2, 128, 3)
    for t in range(2):
        scal[:, t, 0:3] = uwr[t]
        scal[:, t, 3:6] = udr[t]
        scal[:, t, 6] = bias.reshape(2, 128)[t]

    in_maps = []
    for c in range(NCORES):
        slab = np.zeros((C, HS, PLANE), BF16)
        lo, hi = c * HP - 1, c * HP + HP + 1
        s0, s1 = max(lo, 0), min(hi, S)
        slab[:, s0 - lo:HS - (hi - s1)] = xb[:, s0:s1]
        slab = np.ascontiguousarray(slab.reshape(2, 128, HS, PLANE))
        z1 = np.zeros((128, 1), np.float32)
        in_maps.append({"xs": slab, "wts": wts, "scal": scal,
                        "d1": z1, "d2": z1, "d3": z1})
    return in_maps


def kernel(x, U_kh, U_kw, U_kd, U_c_in, U_c_out, bias, _trace=False):
    from concourse.bass_utils import run_bass_kernel_spmd

    if "nc" not in _cache:
        _cache["nc"] = _build_program()
    nc = _cache["nc"]

    in_maps = _host_prep(x, U_kh, U_kw, U_kd, U_c_in, U_c_out, bias)
    res = run_bass_kernel_spmd(nc, in_maps, core_ids=list(range(NCORES)),
                               trace=_trace)
    _cache["last_result"] = res

    out = np.empty((1, CO, S, S, S), np.float32)
    for c in range(NCORES):
        o = res.results[c]["out"]                        # [2, 128, HP, PLANE] bf16
        out[0, :, c * HP:(c + 1) * HP] = o.reshape(CO, HP, S, S).astype(np.float32)
    return out
